# revision 1
# baseline (speedup 1.0000x reference)
"""Causal multi-head attention on 8 Trainium2 cores (raw Bass).

Problem: x[4,2048,1024] @ W_qkv -> 16-head causal attention -> @ W_proj.
Sharding: core c handles batch b=c//2 and head-half c%2 (8 heads each).
Host pre-transposes x (feature-major xT) and pre-slices/scales weights;
each core computes its heads' contribution to out^T; host sums the two
half contributions per batch and adds b_proj.

Per-core pipeline (fp32r matmuls, fp32 PSUM):
  A1: qk^T[f,t] = w_qk^T @ x^T       (q rows pre-scaled by 1/sqrt(dk))
  A2: V[t,f] = x @ w_v (+bias); V_aug has a ones-column per head
  B:  per (head, q-chunk): S^T[k,q] = k^T.T @ q^T on causal blocks,
      P^T = exp(S^T) on ACT, triangle mask on diagonal 128-blocks (DVE),
      y_aug^T = V_aug^T @ P^T accumulated in PSUM (row 64 = softmax sums),
      reciprocal + K=1 replication matmul + DVE multiply to normalize.
      Odd heads staged at partitions 0:64 and DMA-shifted to 64:128.
  C:  out^T = w_proj^T @ y^T, DMA'd out transposed; host transposes back.

build_nc(t, reps) can replicate the whole pipeline `reps` times inside one
NEFF (serialized at rep boundaries) for wall-clock timing dilation.
"""

import contextlib
import math

import numpy as np

import concourse.bass as bass
import concourse.mybir as mybir
from concourse.bass_utils import run_bass_kernel_spmd

F32 = mybir.dt.float32
F32R = mybir.dt.float32r
ADD = mybir.AluOpType.add
MULT = mybir.AluOpType.mult
EXP = mybir.ActivationFunctionType.Exp
COPY = mybir.ActivationFunctionType.Copy

D_MODEL = 1024
D_K = 64
B, T = 4, 2048
NH = 8          # heads per core
KC = 8          # D_MODEL / 128
TQ = 512        # q-chunk width
N_CORES = 8


def build_nc(t=T, reps=1):
    tt_n = t // 128
    tc_n = t // TQ
    nc = bass.Bass(target_bir_lowering=False)

    xT_d = nc.dram_tensor("xT", [128, KC, t], F32R, kind="ExternalInput")
    wqk_d = nc.dram_tensor("wqk", [128, KC, 8, 128], F32R, kind="ExternalInput")
    wv_d = nc.dram_tensor("wv", [128, KC, 512], F32R, kind="ExternalInput")
    wproj_d = nc.dram_tensor("wproj", [128, 4, 1024], F32R, kind="ExternalInput")
    bqk_d = nc.dram_tensor("bqk", [128, 8], F32, kind="ExternalInput")
    bv_d = nc.dram_tensor("bv", [128, 512], F32, kind="ExternalInput")
    tri_d = nc.dram_tensor("tri", [128, 128], F32, kind="ExternalInput")
    ones_d = nc.dram_tensor("onesv", [128, 64], F32R, kind="ExternalInput")
    out_d = nc.dram_tensor("outT", [128, 8, t], F32, kind="ExternalOutput")

    # ---- schedule state ----
    prog = {"sync": [], "tensor": [], "vector": [], "scalar": []}
    cnt = {"pe": 0, "act": 0, "dve": 0}
    for _c in range(8):
        cnt[f"dma{_c}"] = 0
    last_wait = {e: {} for e in prog}
    bank_war = {}          # psum bank -> (sem, value): last consumer finished
    FUSE = {"tensor", "vector", "scalar"}

    def op(engine, fn, waits=(), incs=()):
        w = []
        for s, v in waits:
            if v <= 0 or last_wait[engine].get(s, -1) >= v:
                continue
            last_wait[engine][s] = v
            w.append((s, v))
        prog[engine].append((fn, w, list(incs), engine in FUSE))
        for s, a in incs:
            cnt[s] += a

    NDMA = 8
    dma_rr = [0]

    def dma(dst, src, waits=()):
        ch = dma_rr[0] % NDMA
        dma_rr[0] += 1
        sem = f"dma{ch}"
        w = [(sem, cnt[sem])] + list(waits)   # chain within channel
        op("sync", lambda e, d=dst, s=src: e.dma_start(d, s),
           w, [(sem, 16)])
        return (sem, cnt[sem])

    stack = contextlib.ExitStack()
    sb = lambda name, shape, dt: stack.enter_context(
        nc.sbuf_tensor(name, shape, dt))

    # persistent region
    qk_sb = sb("qk", [128, 8, t], F32R)
    v_sb = sb("vsb", [128, tt_n, 8, 65], F32R)
    bqk_sb = sb("bqk_sb", [128, 8], F32)
    bv_sb = sb("bv_sb", [128, 512], F32)
    tri_sb = sb("tri_sb", [128, 128], F32)
    ones_sb = sb("ones_sb", [128, 64], F32R)
    psum = stack.enter_context(nc.psum_tensor("ps", [128, 8, 512], F32))

    with contextlib.ExitStack() as semstack:
        semstack.enter_context(nc.allow_low_precision(
            reason="fp32r rounding on matmul operands is intentional"))
        sems = {}
        for _nm in ["pe", "act", "dve"] + [f"dma{_c}" for _c in range(8)]:
            sems[_nm] = semstack.enter_context(nc.semaphore(_nm + "_sem"))

        bqk_ret = dma(bqk_sb.ap(), bqk_d[:])
        bv_ret = dma(bv_sb.ap(), bv_d[:])
        tri_ret = dma(tri_sb.ap(), tri_d[:])
        ones_ret = dma(ones_sb.ap(), ones_d[:])

        # ones column of V_aug via DVE (x*0 + 1)
        op("vector",
           lambda e: e.tensor_scalar(
               v_sb.ap()[:, :, :, 64:65],
               bv_sb.ap()[:, 0:tt_n * 8].rearrange(
                   "p (a b c) -> p a b c", a=tt_n, b=8),
               0.0, 1.0, MULT, mybir.AluOpType.add),
           [bv_ret], [("dve", 1)])
        vones_ret = ("dve", cnt["dve"])

        # phase-A region (aliased by phase-B/C tiles; reps serialize fully)
        xa = nc.sbuf_tensor("xT_sb", [128, KC, t], F32R)
        xT_sb = xa.__enter__()
        wqa = nc.sbuf_tensor("wqg", [128, 2, KC, 4, 128], F32R)
        wqg = wqa.__enter__()
        wva = nc.sbuf_tensor("wv_buf", [128, 4, 512], F32R)
        wv_buf = wva.__enter__()
        a_tiles_end = [xa, wqa, wva]

        # reserve phase-B/C tiles now (addresses alias the A region; safe
        # because within a rep B starts only after A's last read, and reps
        # are serialized at the boundary)
        for _a in reversed(a_tiles_end):
            pass  # keep handles; exit later

        first_pv = [True]
        pair_war = {0: 0, 1: 0}
        slot_war = {0: 0, 1: 0}
        ybank_war = {}
        rbank_war = {}
        rsb_war = {}
        ysbt_war = {}
        wqg_last = {}           # kc -> pe cnt of last MM reading wqg[kc]
        pending_tail = []

        def _make_tail(rb, i, recip_done, out_ap, yb, h, g):
            # returns op-tuples for (rep matmul, normalize) of iteration i
            def emit():
                w = [("dve", recip_done), ones_ret]
                if rb in rbank_war:
                    w.append(("dve", rbank_war[rb]))
                op("tensor",
                   lambda e, rb=rb, buf=i % 2: e.matmul(
                       psum.ap()[0:64, rb],
                       ones_sb.ap()[64:65, :],
                       rsb.ap()[64:65, buf],
                       start=True, stop=True),
                   w, [("pe", 1)])
                rep_done = cnt["pe"]
                rsb_war[i % 2] = rep_done
                w = [("pe", rep_done), ("dve", ybank_war[yb])]
                if h % 2 == 1 and (g % 2) in ysbt_war:
                    w.append(ysbt_war[g % 2])
                op("vector",
                   lambda e, o=out_ap, rb=rb, buf=i % 2:
                       e.tensor_tensor(
                           o, yun.ap()[0:64, buf].bitcast(F32),
                           psum.ap()[0:64, rb], MULT),
                   w, [("dve", 1)])
                rbank_war[rb] = cnt["dve"]
            return emit
        c_copy = {}
        c_dma = {}
        out_seq = [0]
        rep_gate = []
        b_alloc = [None]

        for rep in range(reps):
            # ---- phase A1 ----
            xT_done = {}
            wqg_dma = {0: {}, 1: {}}
            if True:
                for kc in range(KC):
                    war = list(rep_gate)
                    if (0, kc) in wqg_last:
                        war.append(("pe", wqg_last[(0, kc)]))
                    wqg_dma[0][kc] = dma(
                        wqg.ap()[:, 0, kc], wqk_d[:, kc, 0:4], war)
                    xT_done[kc] = dma(
                        xT_sb.ap()[:, kc], xT_d[:, kc], list(rep_gate))
                for kc in range(KC):
                    war = list(rep_gate)
                    if (1, kc) in wqg_last:
                        war.append(("pe", wqg_last[(1, kc)]))
                    wqg_dma[1][kc] = dma(
                        wqg.ap()[:, 1, kc], wqk_d[:, kc, 4:8], war)
            for grp in range(2):
                for tc in range(tc_n):
                    for kc in range(KC):
                        for ftl in range(4):
                            bank = (tc % 2) * 4 + ftl
                            w = []
                            if ftl == 0:
                                w = [wqg_dma[grp][kc], xT_done[kc]] + rep_gate
                            if kc == 0 and bank in bank_war:
                                w.append(bank_war.pop(bank))
                            op("tensor",
                               lambda e, b=bank, g_=grp, k=kc, f=ftl, tc_=tc:
                                   e.matmul(
                                       psum.ap()[:, b],
                                       wqg.ap()[:, g_, k, f],
                                       xT_sb.ap()[:, k,
                                                  tc_ * TQ:(tc_ + 1) * TQ],
                                       start=(k == 0), stop=(k == KC - 1)),
                               w, [("pe", 1)] if ftl == 3 else [])
                        if tc == tc_n - 1:
                            wqg_last[(grp, kc)] = cnt["pe"]
                    grp_done = cnt["pe"]
                    for ftl in range(4):
                        ft = grp * 4 + ftl
                        bk = (tc % 2) * 4 + ftl
                        op("vector",
                           lambda e, b=bk, f=ft, tc_=tc:
                               e.tensor_scalar(
                                   qk_sb.ap()[:, f, tc_ * TQ:(tc_ + 1) * TQ],
                                   psum.ap()[:, b],
                                   bqk_sb.ap()[:, f:f + 1], None, ADD),
                           [("pe", grp_done), bqk_ret], [("dve", 1)])
                        bank_war[bk] = ("dve", cnt["dve"])
            a1_copies = cnt["dve"]

            # ---- phase A2 (wv streamed JIT through 4-slot buffer) ----
            tt_groups = [list(range(i, min(i + 4, tt_n)))
                         for i in range(0, tt_n, 4)]
            wv_seq = 0
            a2_kc_done = {}
            for tg, tts in enumerate(tt_groups):
                for kc in range(KC):
                    slot = wv_seq % 4
                    war = list(rep_gate)
                    if wv_seq - 4 >= 0:
                        war.append(("pe", a2_kc_done[wv_seq - 4]))
                    nd = dma(wv_buf.ap()[:, slot], wv_d[:, kc], war)
                    for j, tt in enumerate(tts):
                        bank = (tg % 2) * 4 + j
                        w = [nd] if j == 0 else []
                        if kc == 0 and bank in bank_war:
                            w.append(bank_war.pop(bank))
                        op("tensor",
                           lambda e, b=bank, s=slot, k=kc, tt_=tt:
                               e.matmul(
                                   psum.ap()[:, b],
                                   xT_sb.ap()[:, k, tt_ * 128:(tt_ + 1) * 128],
                                   wv_buf.ap()[:, s],
                                   start=(k == 0), stop=(k == KC - 1)),
                           w, [("pe", 1)] if j == len(tts) - 1 else [])
                    a2_kc_done[wv_seq] = cnt["pe"]
                    wv_seq += 1
                grp_done = cnt["pe"]
                for j, tt in enumerate(tts):
                    bk = (tg % 2) * 4 + j
                    op("vector",
                       lambda e, b=bk, tt_=tt:
                           e.tensor_tensor(
                               v_sb.ap()[:, tt_, :, 0:64],
                               psum.ap()[:, b], bv_sb.ap()[:], ADD),
                       [("pe", grp_done), bv_ret], [("dve", 1)])
                    bank_war[bk] = ("dve", cnt["dve"])
            a2_copies = cnt["dve"]
            a2_pe_done = cnt["pe"]

            if b_alloc[0] is None:
                for _a in reversed(a_tiles_end):
                    _a.__exit__(None, None, None)
                ysb = sb("ysb", [128, 4, t], F32R)
                ysbt = sb("ysbt", [64, 2, t], F32R)
                pt_sb = sb("pt", [128, 4, 512], F32R)
                yun = sb("yun", [64, 2, 512], F32R)
                rsb = sb("rsb", [65, 2, 512], F32R)
                osb = sb("osb", [128, 8, 512], F32)
                wproj_sb = sb("wproj_sb", [128, 4, 1024], F32R)
                b_alloc[0] = (ysb, ysbt, pt_sb, yun, rsb, osb, wproj_sb)
            else:
                ysb, ysbt, pt_sb, yun, rsb, osb, wproj_sb = b_alloc[0]

            wproj_dma = dma(wproj_sb.ap(), wproj_d[:], [("pe", a2_pe_done)])

            # ---- phase B ----
            for h in range(NH):
                g = h // 2
                qrow = (h % 2) * 64
                qf, kf = g, 4 + g
                for qc in range(tc_n):
                    i = h * tc_n + qc
                    yb = 4 + i % 2
                    rb = 6 + i % 2
                    nkt = 4 * qc + 4
                    npairs = 2 * qc + 2

                    def s_mm(kt, bank, qrow=qrow, kf=kf, qf=qf, qc=qc):
                        r = kt - 4 * qc
                        off = max(0, r * 128)
                        n = TQ - off
                        return lambda e, kt=kt, b=bank, off=off, n=n: \
                            e.matmul(
                                psum.ap()[:, b, off:off + n],
                                qk_sb.ap()[qrow:qrow + 64, kf,
                                           kt * 128:(kt + 1) * 128],
                                qk_sb.ap()[qrow:qrow + 64, qf,
                                           qc * TQ + off:qc * TQ + off + n],
                                start=True, stop=True)

                    def pv_mm(kt, slot, start, stop, h=h, qc=qc, yb=yb):
                        r = kt - 4 * qc
                        off = max(0, r * 128)
                        n = TQ - off
                        return lambda e, kt=kt, s=slot, off=off, n=n, \
                            st=start, sp=stop: e.matmul(
                                psum.ap()[0:65, yb, off:off + n],
                                v_sb.ap()[:, kt, h, :],
                                pt_sb.ap()[:, s, off:off + n],
                                start=st, stop=sp)

                    s_done = {}
                    pt_ready = {}

                    for p in range(npairs):
                        pg = p % 2
                        kts = (2 * p, 2 * p + 1)
                        banks = (pg * 2, pg * 2 + 1)
                        w = [("act", pair_war[pg]), ("dve", a1_copies)]
                        if p == 1 and pending_tail:
                            for _t in pending_tail:
                                _t()
                            pending_tail.clear()
                        for bq in banks:
                            if bq in bank_war:
                                w.append(bank_war.pop(bq))
                        op("tensor", s_mm(kts[0], banks[0]), w, [])
                        op("tensor", s_mm(kts[1], banks[1]), [], [("pe", 1)])
                        s_done[p] = cnt["pe"]
                        if p >= 1:
                            pp = p - 1
                            w = [pt_ready[pp]]
                            if first_pv[0]:
                                w += [vones_ret, ("dve", a2_copies)]
                                first_pv[0] = False
                            if pp == 0 and yb in ybank_war:
                                w.append(("dve", ybank_war[yb]))
                            op("tensor",
                               pv_mm(2 * pp, (pp % 2) * 2,
                                     2 * pp == 0, False), w, [])
                            op("tensor",
                               pv_mm(2 * pp + 1, (pp % 2) * 2 + 1, False,
                                     2 * pp + 1 == nkt - 1),
                               [], [("pe", 1)])
                            slot_war[pp % 2] = cnt["pe"]
                        # exp over the whole pair (dead regions of diagonal
                        # blocks hold bounded garbage; PV never reads them)
                        diag = (kts[1] - 4 * qc) >= 0
                        off0 = max(0, (kts[0] - 4 * qc)) * 128
                        w = [("pe", s_done[p]), ("pe", slot_war[pg])]
                        op("scalar",
                           lambda e, bq=banks[0], s=pg * 2, o=off0:
                               e.activation(
                                   pt_sb.ap()[:, s:s + 2]
                                       .rearrange("p a b -> p (a b)")
                                       [:, o:2 * TQ],
                                   psum.ap()[:, bq:bq + 2]
                                       .rearrange("p a b -> p (a b)")
                                       [:, o:2 * TQ],
                                   EXP),
                           w, [("act", 1)])
                        pair_war[pg] = cnt["act"]
                        pt_ready[p] = ("act", cnt["act"])
                        if diag:
                            for j in (0, 1):
                                r = kts[j] - 4 * qc
                                op("vector",
                                   lambda e, s=pg * 2 + j, r=r:
                                       e.tensor_tensor(
                                           pt_sb.ap()[:, s,
                                                      r * 128:r * 128 + 128],
                                           pt_sb.ap()[:, s,
                                                      r * 128:r * 128 + 128],
                                           tri_sb.ap()[:], MULT),
                                   [("act", pt_ready[p][1]), tri_ret],
                                   [("dve", 1)] if j == 1 else [])
                            pt_ready[p] = ("dve", cnt["dve"])

                    pp = npairs - 1
                    w = [pt_ready[pp]]
                    if pp == 0:
                        if first_pv[0]:
                            w += [vones_ret, ("dve", a2_copies)]
                            first_pv[0] = False
                        if yb in ybank_war:
                            w.append(("dve", ybank_war[yb]))
                    op("tensor", pv_mm(2 * pp, (pp % 2) * 2,
                                       2 * pp == 0, False), w, [])
                    op("tensor", pv_mm(2 * pp + 1, (pp % 2) * 2 + 1,
                                       False, True), [], [("pe", 1)])
                    slot_war[pp % 2] = cnt["pe"]
                    pv_all = cnt["pe"]

                    w = [("pe", pv_all)]
                    if i % 2 in rsb_war:
                        w.append(("pe", rsb_war[i % 2]))
                    op("vector",
                       lambda e, yb=yb, buf=i % 2: e.reciprocal(
                           rsb.ap()[64:65, buf], psum.ap()[64:65, yb]),
                       w, [("dve", 1)])
                    recip_done = cnt["dve"]
                    op("vector",
                       lambda e, yb=yb, buf=i % 2: e.tensor_copy(
                           yun.ap()[0:64, buf].bitcast(F32),
                           psum.ap()[0:64, yb]),
                       [], [("dve", 1)])
                    ybank_war[yb] = cnt["dve"]
                    if h % 2 == 0:
                        out_ap = ysb.ap()[0:64, g, qc * TQ:(qc + 1) * TQ]
                    else:
                        out_ap = ysbt.ap()[0:64, g % 2,
                                           qc * TQ:(qc + 1) * TQ]
                    pending_tail.append(_make_tail(
                        rb, i, recip_done, out_ap, yb, h, g))
                if h % 2 == 1:
                    for _t in pending_tail:
                        _t()
                    pending_tail.clear()
                    nd = dma(ysb.ap()[64:128, g], ysbt.ap()[0:64, g % 2],
                             [("dve", cnt["dve"])])
                    ysbt_war[g % 2] = nd
            for _t in pending_tail:
                _t()
            pending_tail.clear()
            b_dve_done = cnt["dve"]
            b_act_done = cnt["act"]
            shift_rets = [ysbt_war[k] for k in ysbt_war]

            # ---- phase C ----
            for tc in range(tc_n):
                for ft in range(8):
                    j = out_seq[0]
                    bank = j % 4
                    w = [("dve", b_dve_done), wproj_dma,
                         ("act", b_act_done)] + shift_rets
                    if j >= 4:
                        w.append(("act", c_copy[j - 4]))
                    for gg in range(4):
                        op("tensor",
                           lambda e, bk=bank, g_=gg, f=ft, tc_=tc: e.matmul(
                               psum.ap()[:, bk],
                               wproj_sb.ap()[:, g_, f * 128:(f + 1) * 128],
                               ysb.ap()[:, g_, tc_ * TQ:(tc_ + 1) * TQ],
                               start=(g_ == 0), stop=(g_ == 3)),
                           w if gg == 0 else [],
                           [("pe", 1)] if gg == 3 else [])
                    mm_done = cnt["pe"]
                    w = [("pe", mm_done)]
                    if j >= 8:
                        w.append(c_dma[j - 8])
                    op("scalar",
                       lambda e, bk=bank, ob=j % 8: e.activation(
                           osb.ap()[:, ob], psum.ap()[:, bk], COPY),
                       w, [("act", 1)])
                    c_copy[j] = cnt["act"]
                    bank_war[bank] = ("act", cnt["act"])
                    c_dma[j] = dma(
                        out_d[:, ft, tc * TQ:(tc + 1) * TQ],
                        osb.ap()[:, j % 8],
                        [("act", c_copy[j])])
                    out_seq[0] += 1
            rep_gate = [("act", c_copy[out_seq[0] - 1]), c_dma[out_seq[0] - 1]]
            # seed psum WARs for next rep's A phase (banks 2,3 were last read
            # by B exps; 4..7 by B's recip/copy/norm)
            bank_war.setdefault(2, ("act", b_act_done))
            bank_war.setdefault(3, ("act", b_act_done))
            for bk in (4, 5):
                bank_war.setdefault(bk, ("dve", ybank_war.get(bk, 0)))
            for bk in (6, 7):
                bank_war.setdefault(bk, ("dve", rbank_war.get(bk, 0)))

        # ---- emit ----
        with nc.Block() as block:
            def emitter(name):
                def run(eng):
                    for fn, waits, incs, fuse in prog[name]:
                        pre = waits[1:] if (fuse and waits) else waits
                        for s, v in pre:
                            eng.wait_ge(sems[s], v)
                        ins = fn(eng)
                        if fuse and waits:
                            s, v = waits[0]
                            ins.wait_op(sems[s], v, "sem-ge")
                        for s, a in incs:
                            ins.then_inc(sems[s], a)
                return run
            block.sync(emitter("sync"))
            block.tensor(emitter("tensor"))
            block.vector(emitter("vector"))
            block.scalar(emitter("scalar"))

    stack.close()
    return nc


# ---------------------------------------------------------------------------

def host_prep(x, W_qkv, b_qkv, W_proj, b_proj, t=T):
    scale = 1.0 / math.sqrt(D_K)
    x = np.asarray(x, np.float32)
    W_qkv = np.asarray(W_qkv, np.float32)
    b_qkv = np.asarray(b_qkv, np.float32)
    W_proj = np.asarray(W_proj, np.float32)

    tri = (np.arange(128)[None, :] >= np.arange(128)[:, None]) \
        .astype(np.float32)
    onesv = np.ones((128, 64), np.float32)

    in_maps = []
    for c in range(N_CORES):
        b = c // 2
        f0 = (c % 2) * 512
        xT = np.ascontiguousarray(
            x[b, :t].T.reshape(KC, 128, t).transpose(1, 0, 2))
        wq = W_qkv[:, f0:f0 + 512] * scale
        wk = W_qkv[:, D_MODEL + f0:D_MODEL + f0 + 512]
        wqk = np.concatenate([wq, wk], axis=1)
        wqk = np.ascontiguousarray(
            wqk.reshape(KC, 128, 8, 128).transpose(1, 0, 2, 3))
        wv = W_qkv[:, 2 * D_MODEL + f0:2 * D_MODEL + f0 + 512]
        wv = np.ascontiguousarray(
            wv.reshape(KC, 128, 512).transpose(1, 0, 2))
        bq = b_qkv[f0:f0 + 512] * scale
        bk = b_qkv[D_MODEL + f0:D_MODEL + f0 + 512]
        bqk = np.ascontiguousarray(
            np.concatenate([bq, bk]).reshape(8, 128).T)
        bv = b_qkv[2 * D_MODEL + f0:2 * D_MODEL + f0 + 512]
        bv_rep = np.broadcast_to(bv, (128, 512)).copy()
        wp = W_proj[f0:f0 + 512]
        wp = np.ascontiguousarray(
            wp.reshape(4, 128, 1024).transpose(1, 0, 2))
        in_maps.append({
            "xT": xT, "wqk": wqk, "wv": wv, "wproj": wp,
            "bqk": bqk, "bv": bv_rep, "tri": tri, "onesv": onesv,
        })
    return in_maps


def host_gather(results, b_proj, t=T):
    b_proj = np.asarray(b_proj, np.float32)
    out = np.empty((B, t, D_MODEL), np.float32)
    for b in range(B):
        acc = None
        for half in range(2):
            r = results[2 * b + half]["outT"]
            oT = r.transpose(1, 0, 2).reshape(D_MODEL, t)
            acc = oT if acc is None else acc + oT
        out[b] = acc.T + b_proj
    return out


_NC_CACHE = {}


def kernel(x, W_qkv, b_qkv, W_proj, b_proj):
    if T not in _NC_CACHE:
        _NC_CACHE[T] = build_nc(T)
    nc = _NC_CACHE[T]
    in_maps = host_prep(x, W_qkv, b_qkv, W_proj, b_proj)
    res = run_bass_kernel_spmd(nc, in_maps, core_ids=list(range(N_CORES)))
    return host_gather(res.results, b_proj)



# revision 7
# speedup vs baseline: 1.0674x; 1.0674x over previous
"""Causal multi-head attention on 8 Trainium2 cores (raw Bass).

Problem: x[4,2048,1024] @ W_qkv -> 16-head causal attention -> @ W_proj.
Sharding: core c handles batch b=c//2 and head-half c%2 (8 heads each).
Host pre-transposes x (feature-major xT) and pre-slices/scales weights;
each core computes its heads' contribution to out^T; host sums the two
half contributions per batch and adds b_proj.

Per-core pipeline (bf16/fp8 matmuls, fp32 PSUM):
  A1: qk^T[f,t] = w_qk^T @ x^T via fp8e4m3 DoubleRow 3-term compensated
      matmuls (hi*hi + x*lo + lo*xhat), per-block power-of-2 scales
      descaled in the DVE bias-add epilogue -> qk_sb bf16.
  A2: V[t,f] = x @ w_v same fp8 DR scheme (+bias); V_aug ones-column.
  B:  per (head, q-chunk): S^T[k,q] = k^T.T @ q^T (bf16) on causal
      blocks, P^T = exp(S^T) on ACT -> bf16, triangle mask on diagonal
      128-blocks (DVE), y_aug^T = V_aug^T @ P^T in PSUM (row 64 = sums),
      reciprocal (DVE) -> DMA sbuf broadcast [1,512]->[64,512] -> DVE
      multiply to normalize. Odd heads staged and DMA-shifted to 64:128.
  C:  out^T = w_proj^T @ y^T (bf16), per-g-block dependency waits so PE
      flows from B's last PV directly into C; out stored bf16, host
      upcasts/sums.

build_nc(t, reps) can replicate the pipeline `reps` times in one NEFF
(serialized at rep boundaries) for wall-clock timing dilation.
"""

import contextlib
import math

import numpy as np
import ml_dtypes

import concourse.bass as bass
import concourse.mybir as mybir
from concourse.bass_utils import run_bass_kernel_spmd

F32 = mybir.dt.float32
BF16 = mybir.dt.bfloat16
F8 = mybir.dt.float8e4
ADD = mybir.AluOpType.add
MULT = mybir.AluOpType.mult
EXP = mybir.ActivationFunctionType.Exp
COPY = mybir.ActivationFunctionType.Copy
DRM = mybir.MatmulPerfMode.DoubleRow

NBF = ml_dtypes.bfloat16
NF8 = ml_dtypes.float8_e4m3

D_MODEL = 1024
D_K = 64
B, T = 4, 2048
NH = 8          # heads per core
KC = 8          # D_MODEL / 128
TQ = 512        # q-chunk width
N_CORES = 8

A_SC = 16.0     # x hi scale
R_SC = 16.0     # residual boost
BQ_SC = 256.0   # w scale, q features (pre-scaled by 1/sqrt(dk))
BK_SC = 32.0    # w scale, k features
BV_SC = 32.0    # w scale, v features
HEAD_ORDER = [1, 0, 3, 2, 5, 4, 7, 6]   # odd first so shifts overlap
VARS = "hpl"
# (w variant, x variant) term pairs: hi*hi, lo*xhat, xlo*what
A_TERMS = [("h", "h"), ("l", "p"), ("p", "l")]


def build_nc(t=T, reps=1):
    tt_n = t // 128
    tc_n = t // TQ
    nc = bass.Bass(target_bir_lowering=False)

    x_d = {v: nc.dram_tensor(f"x{v}", [128, KC, t], F8,
                             kind="ExternalInput") for v in VARS}
    wqk_d = {v: nc.dram_tensor(f"wqk{v}", [128, KC, 8, 128], F8,
                               kind="ExternalInput") for v in VARS}
    wv_d = {v: nc.dram_tensor(f"wv{v}", [128, KC, 512], F8,
                              kind="ExternalInput") for v in VARS}
    wproj_d = nc.dram_tensor("wproj", [128, 4, 1024], BF16,
                             kind="ExternalInput")
    bqk_d = nc.dram_tensor("bqk", [128, 8], F32, kind="ExternalInput")
    bv_d = nc.dram_tensor("bv", [128, 512], F32, kind="ExternalInput")
    tri_d = nc.dram_tensor("tri", [128, 128], BF16, kind="ExternalInput")
    out_d = nc.dram_tensor("outT", [128, 8, t], BF16, kind="ExternalOutput")

    # ---- schedule state ----
    prog = {"sync": [], "tensor": [], "vector": [], "scalar": []}
    cnt = {"pe": 0, "act": 0, "dve": 0}
    for _c in range(8):
        cnt[f"dma{_c}"] = 0
    last_wait = {e: {} for e in prog}
    bank_war = {}          # psum bank -> (sem, value): last consumer done
    FUSE = {"tensor", "vector", "scalar"}

    def op(engine, fn, waits=(), incs=()):
        w = []
        for s, v in waits:
            if v <= 0 or last_wait[engine].get(s, -1) >= v:
                continue
            last_wait[engine][s] = v
            w.append((s, v))
        prog[engine].append((fn, w, list(incs), engine in FUSE))
        for s, a in incs:
            cnt[s] += a

    NDMA = 8
    dma_rr = [0]

    def dma(dst, src, waits=()):
        ch = dma_rr[0] % NDMA
        dma_rr[0] += 1
        sem = f"dma{ch}"
        w = [(sem, cnt[sem])] + list(waits)   # chain within channel
        op("sync", lambda e, d=dst, s=src: e.dma_start(d, s),
           w, [(sem, 16)])
        return (sem, cnt[sem])

    stack = contextlib.ExitStack()
    sb = lambda name, shape, dt: stack.enter_context(
        nc.sbuf_tensor(name, shape, dt))

    # persistent region
    qk_sb = sb("qk", [128, 8, t], BF16)
    v_sb = sb("vsb", [128, tt_n, 8, 65], BF16)
    bqk_sb = sb("bqk_sb", [128, 8], F32)
    bv_sb = sb("bv_sb", [128, 512], F32)
    tri_sb = sb("tri_sb", [128, 128], BF16)
    psum = stack.enter_context(nc.psum_tensor("ps", [128, 8, 512], F32))

    with contextlib.ExitStack() as semstack:
        semstack.enter_context(nc.allow_low_precision(
            reason="bf16/fp8 matmul operands and bf16 recip intentional"))
        sems = {}
        for _nm in ["pe", "act", "dve"] + [f"dma{_c}" for _c in range(8)]:
            sems[_nm] = semstack.enter_context(nc.semaphore(_nm + "_sem"))

        bqk_ret = dma(bqk_sb.ap(), bqk_d[:])
        bv_ret = dma(bv_sb.ap(), bv_d[:])
        tri_ret = dma(tri_sb.ap(), tri_d[:])

        # ones column of V_aug via DVE (x*0 + 1)
        op("vector",
           lambda e: e.tensor_scalar(
               v_sb.ap()[:, :, :, 64:65],
               bv_sb.ap()[:, 0:tt_n * 8].rearrange(
                   "p (a b c) -> p a b c", a=tt_n, b=8),
               0.0, 1.0, MULT, mybir.AluOpType.add),
           [bv_ret], [("dve", 1)])
        vones_ret = ("dve", cnt["dve"])

        # phase-A region (aliased by phase-B/C tiles; reps serialize fully)
        a_ctx = []

        def a_sb(name, shape, dt):
            c = nc.sbuf_tensor(name, shape, dt)
            hdl = c.__enter__()
            a_ctx.append(c)
            return hdl

        x_sb = {v: a_sb(f"x{v}_sb", [128, KC, t], F8) for v in VARS}
        wqk_sb = {v: a_sb(f"wqk{v}_sb", [128, KC, 8, 128], F8)
                  for v in VARS}
        wv_sb = {v: a_sb(f"wv{v}_sb", [128, KC, 512], F8) for v in VARS}

        first_pv = [True]
        pair_war = {0: 0, 1: 0}
        slot_war = {0: 0, 1: 0}
        ybank_war = {}
        rsb_war = {}        # buf -> bcast dma ret (WAR for recip write)
        rep_war = {}        # buf -> dve cnt of norm (WAR for bcast write)
        ysbt_war = {}
        pending_tail = []
        b_alloc = [None]
        c_copy = {}
        c_dma = {}
        out_seq = [0]
        rep_gate = []

        for rep in range(reps):
            # ---- phase A DMAs (term-major: T1's tensors land first) ----
            xdma = {v: {} for v in VARS}
            wqdma = {v: {} for v in VARS}
            wvdma = {v: {} for v in VARS}
            for wvar, xvar in A_TERMS:
                for kc in range(KC):
                    xdma[xvar][kc] = dma(
                        x_sb[xvar].ap()[:, kc], x_d[xvar][:, kc],
                        list(rep_gate))
                    wqdma[wvar][kc] = dma(
                        wqk_sb[wvar].ap()[:, kc], wqk_d[wvar][:, kc],
                        list(rep_gate))
            for wvar, _ in A_TERMS:
                for kc in range(KC):
                    wvdma[wvar][kc] = dma(
                        wv_sb[wvar].ap()[:, kc], wv_d[wvar][:, kc],
                        list(rep_gate))

            # ---- phase A1: qk^T = w_qk^T @ x^T (fp8 DR 3-term) ----
            for grp in range(2):
                for tc in range(tc_n):
                    done = {}
                    for ti, (wvar, xvar) in enumerate(A_TERMS):
                        for kp in range(4):
                            for ftl in range(4):
                                bank = (tc % 2) * 4 + ftl
                                ft = grp * 4 + ftl
                                w = list(rep_gate)
                                if ftl == 0:
                                    w += [xdma[xvar][2 * kp],
                                          xdma[xvar][2 * kp + 1],
                                          wqdma[wvar][2 * kp],
                                          wqdma[wvar][2 * kp + 1]]
                                if kp == 0 and ti == 0 \
                                        and bank in bank_war:
                                    w.append(bank_war.pop(bank))
                                last = (kp == 3 and ti == 2)
                                op("tensor",
                                   lambda e, b=bank, f=ft, k=kp, tc_=tc,
                                   wv_=wvar, xv_=xvar, st=(
                                       kp == 0 and ti == 0), sp=last:
                                       e.matmul(
                                           psum.ap()[:, b],
                                           wqk_sb[wv_].ap()[
                                               :, 2 * k:2 * k + 2, f],
                                           x_sb[xv_].ap()[
                                               :, 2 * k:2 * k + 2,
                                               tc_ * TQ:(tc_ + 1) * TQ],
                                           start=st, stop=sp,
                                           perf_mode=DRM),
                                   w, [("pe", 1)] if last else [])
                                if last:
                                    done[ftl] = cnt["pe"]
                    dsc = 1.0 / (A_SC * (BQ_SC if grp == 0 else BK_SC))
                    for ftl in range(4):
                        ft = grp * 4 + ftl
                        bk = (tc % 2) * 4 + ftl
                        op("vector",
                           lambda e, b=bk, f=ft, tc_=tc, c=dsc:
                               e.tensor_scalar(
                                   qk_sb.ap()[:, f, tc_ * TQ:(tc_ + 1) * TQ],
                                   psum.ap()[:, b],
                                   c, bqk_sb.ap()[:, f:f + 1], MULT, ADD),
                           [("pe", done[ftl]), bqk_ret], [("dve", 1)])
                        bank_war[bk] = ("dve", cnt["dve"])
            a1_copies = cnt["dve"]

            # ---- phase A2: V = x @ w_v (fp8 DR 3-term) ----
            tt_groups = [list(range(i, min(i + 4, tt_n)))
                         for i in range(0, tt_n, 4)]
            for tg, tts in enumerate(tt_groups):
                done = {}
                for ti, (wvar, xvar) in enumerate(A_TERMS):
                    for kp in range(4):
                        for j, tt in enumerate(tts):
                            bank = (tg % 2) * 4 + j
                            w = list(rep_gate)
                            if j == 0:
                                w += [wvdma[wvar][2 * kp],
                                      wvdma[wvar][2 * kp + 1],
                                      xdma[xvar][2 * kp],
                                      xdma[xvar][2 * kp + 1]]
                            if kp == 0 and ti == 0 \
                                    and bank in bank_war:
                                w.append(bank_war.pop(bank))
                            last = (kp == 3 and ti == 2)
                            op("tensor",
                               lambda e, b=bank, k=kp, tt_=tt, wv_=wvar,
                               xv_=xvar, st=(
                                   kp == 0 and (wvar, xvar) == A_TERMS[0]),
                               sp=last:
                                   e.matmul(
                                       psum.ap()[:, b],
                                       x_sb[xv_].ap()[
                                           :, 2 * k:2 * k + 2,
                                           tt_ * 128:(tt_ + 1) * 128],
                                       wv_sb[wv_].ap()[:, 2 * k:2 * k + 2],
                                       start=st, stop=sp,
                                       perf_mode=DRM),
                               w, [("pe", 1)] if last else [])
                            if last:
                                done[j] = cnt["pe"]
                for j, tt in enumerate(tts):
                    bk = (tg % 2) * 4 + j
                    op("vector",
                       lambda e, b=bk, tt_=tt:
                           e.scalar_tensor_tensor(
                               v_sb.ap()[:, tt_, :, 0:64],
                               psum.ap()[:, b], 1.0 / (A_SC * BV_SC),
                               bv_sb.ap()[:], MULT, ADD),
                       [("pe", done[j]), bv_ret], [("dve", 1)])
                    bank_war[bk] = ("dve", cnt["dve"])
            a2_copies = cnt["dve"]
            a2_pe_done = cnt["pe"]

            if b_alloc[0] is None:
                for _a in reversed(a_ctx):
                    _a.__exit__(None, None, None)
                ysb = sb("ysb", [128, 4, t], BF16)
                ysbt = sb("ysbt", [64, 2, t], BF16)
                pt_sb = sb("pt", [128, 4, 512], BF16)
                yun = sb("yun", [64, 2, 512], BF16)
                rsb = sb("rsb", [65, 2, 512], BF16)
                rep_sb = sb("repb", [64, 2, 512], BF16)
                osb = sb("osb", [128, 8, 512], BF16)
                wproj_sb = sb("wproj_sb", [128, 4, 1024], BF16)
                b_alloc[0] = (ysb, ysbt, pt_sb, yun, rsb, rep_sb, osb,
                              wproj_sb)
            else:
                (ysb, ysbt, pt_sb, yun, rsb, rep_sb, osb,
                 wproj_sb) = b_alloc[0]

            wproj_dma = dma(wproj_sb.ap(), wproj_d[:],
                            [("pe", a2_pe_done)])

            # ---- phase B ----
            ydone = {}        # g -> list of waits for phase C
            for hi, h in enumerate(HEAD_ORDER):
                g = h // 2
                qrow = (h % 2) * 64
                qf, kf = g, 4 + g
                for qc in range(tc_n):
                    i = hi * tc_n + qc
                    yb = 4 + i % 2
                    nkt = 4 * qc + 4
                    npairs = 2 * qc + 2

                    def s_mm(kt, bank, qrow=qrow, kf=kf, qf=qf, qc=qc):
                        r = kt - 4 * qc
                        off = max(0, r * 128)
                        n = TQ - off
                        return lambda e, kt=kt, b=bank, off=off, n=n: \
                            e.matmul(
                                psum.ap()[:, b, off:off + n],
                                qk_sb.ap()[qrow:qrow + 64, kf,
                                           kt * 128:(kt + 1) * 128],
                                qk_sb.ap()[qrow:qrow + 64, qf,
                                           qc * TQ + off:qc * TQ + off + n],
                                start=True, stop=True)

                    def pv_mm(kt, slot, start, stop, h=h, qc=qc, yb=yb):
                        r = kt - 4 * qc
                        off = max(0, r * 128)
                        n = TQ - off
                        return lambda e, kt=kt, s=slot, off=off, n=n, \
                            st=start, sp=stop: e.matmul(
                                psum.ap()[0:65, yb, off:off + n],
                                v_sb.ap()[:, kt, h, :],
                                pt_sb.ap()[:, s, off:off + n],
                                start=st, stop=sp)

                    s_done = {}
                    pt_ready = {}

                    for p in range(npairs):
                        pg = p % 2
                        kts = (2 * p, 2 * p + 1)
                        banks = (pg * 2, pg * 2 + 1)
                        w = [("act", pair_war[pg]), ("dve", a1_copies)]
                        if p == 1 and pending_tail:
                            for _t in pending_tail:
                                _t()
                            pending_tail.clear()
                        for bq in banks:
                            if bq in bank_war:
                                w.append(bank_war.pop(bq))
                        op("tensor", s_mm(kts[0], banks[0]), w, [])
                        op("tensor", s_mm(kts[1], banks[1]), [],
                           [("pe", 1)])
                        s_done[p] = cnt["pe"]
                        if p >= 1:
                            pp = p - 1
                            w = [pt_ready[pp]]
                            if first_pv[0]:
                                w += [vones_ret, ("dve", a2_copies)]
                                first_pv[0] = False
                            if pp == 0 and yb in ybank_war:
                                w.append(("dve", ybank_war[yb]))
                            op("tensor",
                               pv_mm(2 * pp, (pp % 2) * 2,
                                     2 * pp == 0, False), w, [])
                            op("tensor",
                               pv_mm(2 * pp + 1, (pp % 2) * 2 + 1, False,
                                     2 * pp + 1 == nkt - 1),
                               [], [("pe", 1)])
                            slot_war[pp % 2] = cnt["pe"]
                        # exp; diag pair split to skip dead columns
                        diag = (kts[1] - 4 * qc) >= 0
                        r0 = kts[0] - 4 * qc
                        w = [("pe", s_done[p]), ("pe", slot_war[pg])]
                        if diag and r0 >= 2:
                            # two exps: bank r0 cols [128*r0:512],
                            # bank r0+1 cols [128*(r0+1):512]
                            op("scalar",
                               lambda e, bq=banks[0], s=pg * 2, o=128 * r0:
                                   e.activation(
                                       pt_sb.ap()[:, s, o:TQ],
                                       psum.ap()[:, bq, o:TQ], EXP),
                               w, [])
                            op("scalar",
                               lambda e, bq=banks[1], s=pg * 2 + 1,
                               o=128 * (r0 + 1):
                                   e.activation(
                                       pt_sb.ap()[:, s, o:TQ],
                                       psum.ap()[:, bq, o:TQ], EXP),
                               [], [("act", 1)])
                        else:
                            off0 = max(0, r0) * 128
                            op("scalar",
                               lambda e, bq=banks[0], s=pg * 2, o=off0:
                                   e.activation(
                                       pt_sb.ap()[:, s:s + 2]
                                           .rearrange("p a b -> p (a b)")
                                           [:, o:2 * TQ],
                                       psum.ap()[:, bq:bq + 2]
                                           .rearrange("p a b -> p (a b)")
                                           [:, o:2 * TQ],
                                       EXP),
                               w, [("act", 1)])
                        pair_war[pg] = cnt["act"]
                        pt_ready[p] = ("act", cnt["act"])
                        if diag:
                            for j in (0, 1):
                                r = kts[j] - 4 * qc
                                if r < 0:
                                    continue
                                op("vector",
                                   lambda e, s=pg * 2 + j, r=r:
                                       e.tensor_tensor(
                                           pt_sb.ap()[:, s,
                                                      r * 128:r * 128 + 128],
                                           pt_sb.ap()[:, s,
                                                      r * 128:r * 128 + 128],
                                           tri_sb.ap()[:], MULT),
                                   [("act", pt_ready[p][1]), tri_ret],
                                   [("dve", 1)] if j == 1 else [])
                            pt_ready[p] = ("dve", cnt["dve"])

                    pp = npairs - 1
                    w = [pt_ready[pp]]
                    if pp == 0:
                        if first_pv[0]:
                            w += [vones_ret, ("dve", a2_copies)]
                            first_pv[0] = False
                        if yb in ybank_war:
                            w.append(("dve", ybank_war[yb]))
                    op("tensor", pv_mm(2 * pp, (pp % 2) * 2,
                                       2 * pp == 0, False), w, [])
                    op("tensor", pv_mm(2 * pp + 1, (pp % 2) * 2 + 1,
                                       False, True), [], [("pe", 1)])
                    slot_war[pp % 2] = cnt["pe"]
                    pv_all = cnt["pe"]

                    buf = i % 2
                    # reciprocal of sums (row 64) -> bf16
                    w = [("pe", pv_all)]
                    if buf in rsb_war:
                        w.append(rsb_war[buf])
                    op("vector",
                       lambda e, yb=yb, b=buf: e.reciprocal(
                           rsb.ap()[64:65, b], psum.ap()[64:65, yb]),
                       w, [("dve", 1)])
                    recip_done = cnt["dve"]
                    op("vector",
                       lambda e, yb=yb, b=buf: e.tensor_copy(
                           yun.ap()[0:64, b], psum.ap()[0:64, yb]),
                       [], [("dve", 1)])
                    ybank_war[yb] = cnt["dve"]
                    # broadcast recip row across 64 partitions via DMA
                    bw = [("dve", recip_done)]
                    if buf in rep_war:
                        bw.append(("dve", rep_war[buf]))
                    bsrc = rsb.ap()[64:65, buf] \
                        .rearrange("p (a n) -> p a n", a=1) \
                        .broadcast_to([1, 64, TQ])
                    bcast_ret = dma(rep_sb.ap()[0:64, buf], bsrc, bw)
                    rsb_war[buf] = bcast_ret

                    if h % 2 == 0:
                        out_ap = ysb.ap()[0:64, g, qc * TQ:(qc + 1) * TQ]
                    else:
                        out_ap = ysbt.ap()[0:64, g % 2,
                                           qc * TQ:(qc + 1) * TQ]

                    def _tail(out_ap=out_ap, buf=buf, bret=bcast_ret,
                              h=h, g=g):
                        def emit():
                            w = [bret]
                            if h % 2 == 1 and (g % 2) in ysbt_war:
                                w.append(ysbt_war[g % 2])
                            op("vector",
                               lambda e, o=out_ap, b=buf:
                                   e.tensor_tensor(
                                       o, yun.ap()[0:64, b],
                                       rep_sb.ap()[0:64, b], MULT),
                               w, [("dve", 1)])
                            rep_war[buf] = cnt["dve"]
                        return emit
                    pending_tail.append(_tail())

                if h % 2 == 1:
                    # odd head processed first in its pair: emit its tail
                    # now, then DMA-shift staging into ysb rows 64:128
                    for _t in pending_tail:
                        _t()
                    pending_tail.clear()
                    nd = dma(ysb.ap()[64:128, g], ysbt.ap()[0:64, g % 2],
                             [("dve", cnt["dve"])])
                    ysbt_war[g % 2] = nd
                    ydone.setdefault(g, []).append(nd)
                else:
                    # even head: tails may still be pending; flush so the
                    # g block is complete before C consumes it
                    for _t in pending_tail:
                        _t()
                    pending_tail.clear()
                    ydone.setdefault(g, []).append(("dve", cnt["dve"]))
            b_act_done = cnt["act"]

            # ---- phase C (per-g waits; first tiles defer g3) ----
            def c_mm(bank, g_, ft, tc_, st, sp):
                return lambda e, bk=bank, g=g_, f=ft, tc__=tc_, s=st, p=sp: \
                    e.matmul(
                        psum.ap()[:, bk],
                        wproj_sb.ap()[:, g, f * 128:(f + 1) * 128],
                        ysb.ap()[:, g, tc__ * TQ:(tc__ + 1) * TQ],
                        start=s, stop=p)

            def c_tile_start(j, bank, ft, tc_):
                w = [wproj_dma, ("act", b_act_done)]
                if j >= 4:
                    w.append(("act", c_copy[j - 4]))
                if bank in bank_war:
                    w.append(bank_war.pop(bank))
                for g_ in range(3):
                    op("tensor", c_mm(bank, g_, ft, tc_, g_ == 0, False),
                       (w + ydone[g_]) if g_ == 0 else ydone[g_], [])

            def c_tile_end(j, bank, ft, tc_):
                op("tensor", c_mm(bank, 3, ft, tc_, False, True),
                   ydone[3], [("pe", 1)])
                mm_done = cnt["pe"]
                w = [("pe", mm_done)]
                if j >= 8:
                    w.append(c_dma[j - 8])
                op("scalar",
                   lambda e, bk=bank, ob=j % 8: e.activation(
                       osb.ap()[:, ob], psum.ap()[:, bk], COPY),
                   w, [("act", 1)])
                c_copy[j] = cnt["act"]
                bank_war[bank] = ("act", cnt["act"])
                c_dma[j] = dma(
                    out_d[:, ft, tc_ * TQ:(tc_ + 1) * TQ],
                    osb.ap()[:, j % 8],
                    [("act", c_copy[j])])

            tiles = [(tc, ft) for tc in range(tc_n) for ft in range(8)]
            defer = 2
            pend = []
            for idx, (tc, ft) in enumerate(tiles):
                j = out_seq[0] + idx
                bank = j % 4
                c_tile_start(j, bank, ft, tc)
                if idx < defer:
                    pend.append((j, bank, ft, tc))
                    continue
                while pend:
                    c_tile_end(*pend.pop(0))
                c_tile_end(j, bank, ft, tc)
            while pend:
                c_tile_end(*pend.pop(0))
            out_seq[0] += len(tiles)
            rep_gate = [("act", c_copy[out_seq[0] - 1]),
                        c_dma[out_seq[0] - 1]]
            # seed psum WARs for next rep's A phase
            for bk in (4, 5):
                bank_war.setdefault(bk, ("dve", ybank_war.get(bk, 0)))

        # ---- emit ----
        with nc.Block() as block:
            def emitter(name):
                def run(eng):
                    for fn, waits, incs, fuse in prog[name]:
                        pre = waits[1:] if (fuse and waits) else waits
                        for s, v in pre:
                            eng.wait_ge(sems[s], v)
                        ins = fn(eng)
                        if fuse and waits:
                            s, v = waits[0]
                            ins.wait_op(sems[s], v, "sem-ge")
                        for s, a in incs:
                            ins.then_inc(sems[s], a)
                return run
            block.sync(emitter("sync"))
            block.tensor(emitter("tensor"))
            block.vector(emitter("vector"))
            block.scalar(emitter("scalar"))

    stack.close()
    return nc


# ---------------------------------------------------------------------------

def _f8(v):
    return np.ascontiguousarray(v).astype(NF8)


def _split8(v, hi_sc, r_sc=R_SC):
    """Return (hi, hat, lo) fp8 arrays for scaled 3-term matmul."""
    hi = _f8(hi_sc * v)
    lo = _f8(r_sc * (hi_sc * v - hi.astype(np.float32)))
    hat = _f8((hi_sc / r_sc) * v)
    return hi, hat, lo


def host_prep(x, W_qkv, b_qkv, W_proj, b_proj, t=T):
    scale = 1.0 / math.sqrt(D_K)
    x = np.asarray(x, np.float32)
    W_qkv = np.asarray(W_qkv, np.float32)
    b_qkv = np.asarray(b_qkv, np.float32)
    W_proj = np.asarray(W_proj, np.float32)

    tri = (np.arange(128)[None, :] >= np.arange(128)[:, None]) \
        .astype(NBF)

    in_maps = []
    for c in range(N_CORES):
        b = c // 2
        f0 = (c % 2) * 512
        xT = np.ascontiguousarray(
            x[b, :t].T.reshape(KC, 128, t).transpose(1, 0, 2))
        xh, xp, xl = _split8(xT, A_SC)

        wq = W_qkv[:, f0:f0 + 512] * scale
        wk = W_qkv[:, D_MODEL + f0:D_MODEL + f0 + 512]

        def wlayout(w):  # [1024, 512] -> [128, KC, 4, 128]
            return np.ascontiguousarray(
                w.reshape(KC, 128, 4, 128).transpose(1, 0, 2, 3))

        wqk = {}
        qh, qp, ql = _split8(wlayout(wq), BQ_SC)
        kh, kp_, kl = _split8(wlayout(wk), BK_SC)
        wqk["h"] = np.concatenate([qh, kh], axis=2)
        wqk["p"] = np.concatenate([qp, kp_], axis=2)
        wqk["l"] = np.concatenate([ql, kl], axis=2)

        wv = W_qkv[:, 2 * D_MODEL + f0:2 * D_MODEL + f0 + 512]
        wv = np.ascontiguousarray(
            wv.reshape(KC, 128, 512).transpose(1, 0, 2))
        vh, vp, vl = _split8(wv, BV_SC)

        bq = b_qkv[f0:f0 + 512] * scale
        bk_ = b_qkv[D_MODEL + f0:D_MODEL + f0 + 512]
        bqk = np.ascontiguousarray(
            np.concatenate([bq, bk_]).reshape(8, 128).T).astype(np.float32)
        bv = b_qkv[2 * D_MODEL + f0:2 * D_MODEL + f0 + 512]
        bv_rep = np.broadcast_to(bv, (128, 512)).astype(np.float32).copy()
        wp = W_proj[f0:f0 + 512]
        wp = np.ascontiguousarray(
            wp.reshape(4, 128, 1024).transpose(1, 0, 2)).astype(NBF)
        in_maps.append({
            "xh": xh, "xp": xp, "xl": xl,
            "wqkh": wqk["h"], "wqkp": wqk["p"], "wqkl": wqk["l"],
            "wvh": vh, "wvp": vp, "wvl": vl,
            "wproj": wp, "bqk": bqk, "bv": bv_rep, "tri": tri,
        })
    return in_maps


def host_gather(results, b_proj, t=T):
    b_proj = np.asarray(b_proj, np.float32)
    out = np.empty((B, t, D_MODEL), np.float32)
    for b in range(B):
        acc = None
        for half in range(2):
            r = results[2 * b + half]["outT"].astype(np.float32)
            oT = r.transpose(1, 0, 2).reshape(D_MODEL, t)
            acc = oT if acc is None else acc + oT
        out[b] = acc.T + b_proj
    return out


_NC_CACHE = {}


def kernel(x, W_qkv, b_qkv, W_proj, b_proj):
    if T not in _NC_CACHE:
        _NC_CACHE[T] = build_nc(T)
    nc = _NC_CACHE[T]
    in_maps = host_prep(x, W_qkv, b_qkv, W_proj, b_proj)
    res = run_bass_kernel_spmd(nc, in_maps, core_ids=list(range(N_CORES)))
    return host_gather(res.results, b_proj)


# revision 17
# speedup vs baseline: 1.2874x; 1.2061x over previous
"""Causal multi-head attention on 8 Trainium2 cores (raw Bass).

Problem: x[4,2048,1024] @ W_qkv -> 16-head causal attention -> @ W_proj.
Sharding: core c handles batch b=c//2 and head-half c%2 (8 heads each).
Host pre-transposes x (feature-major xT) and pre-slices/scales weights;
each core computes its heads' contribution to out^T; host sums the two
half contributions per batch and adds b_proj.

Per-core pipeline (bf16/fp8 matmuls, fp32 PSUM), wavefront schedule:
  A1: qk^T[f,t] = w_qk^T @ x^T via fp8e4m3 DoubleRow 3-term compensated
      matmuls (hi*hi + xhat*wlo + xlo*what), per-block power-of-2 scales
      descaled in the DVE bias-add epilogue -> qk_sb bf16.
  Then four waves, one per 512-wide q/t chunk qc:
    B(all heads, qc): S^T = k^T.T @ q^T (bf16) on causal blocks,
      P^T = exp(S^T) on ACT -> bf16 (diag pair split to skip dead cols),
      triangle mask on diagonal 128-blocks (DVE), y_aug^T = V_aug^T @ P^T
      in PSUM (row 64 = sums), reciprocal (DVE) -> DMA sbuf broadcast
      [1,512]->[64,512] -> DVE multiply to normalize.  Odd heads staged
      per-chunk and DMA-shifted into ysb rows 64:128.
    A2(tg qc+1): V[t,f] = x @ w_v for the next wave's 4 t-tiles (fp8 DR
      3-term, psum banks 6/7) -- fills PE while B's exp tail drains.
    C(tc=qc): out^T = w_proj^T @ y^T (bf16) for this wave's q columns.
  Interleaving keeps the scalar engine's exp stream off the critical
  path: each wave has more PE work than ACT work.

build_nc(t, reps) can replicate the pipeline `reps` times in one NEFF
(serialized at rep boundaries) for wall-clock timing dilation.
"""

import contextlib
import math

import numpy as np
import ml_dtypes

import concourse.bass as bass
import concourse.mybir as mybir
from concourse.bass_utils import run_bass_kernel_spmd

F32 = mybir.dt.float32
BF16 = mybir.dt.bfloat16
F8 = mybir.dt.float8e4
ADD = mybir.AluOpType.add
MULT = mybir.AluOpType.mult
EXP = mybir.ActivationFunctionType.Exp
COPY = mybir.ActivationFunctionType.Copy
DRM = mybir.MatmulPerfMode.DoubleRow

NBF = ml_dtypes.bfloat16
NF8 = ml_dtypes.float8_e4m3

D_MODEL = 1024
D_K = 64
B, T = 4, 2048
NH = 8          # heads per core
KC = 8          # D_MODEL / 128
TQ = 512        # q-chunk width
N_CORES = 8

A_SC = 16.0     # x hi scale
BQ_SC = 256.0   # w scale, q features (pre-scaled by 1/sqrt(dk))
BK_SC = 32.0    # w scale, k features
BV_SC = 32.0    # w scale, v features
VARS = "hl"
# (w variant, x variant) term pairs: hi*hi + hi*lo + lo*hi
A_TERMS = [("h", "h"), ("l", "h"), ("h", "l")]


def build_nc(t=T, reps=1):
    tt_n = t // 128
    tc_n = t // TQ
    nc = bass.Bass(target_bir_lowering=False)

    x_d = {v: nc.dram_tensor(f"x{v}", [128, KC, t], F8,
                             kind="ExternalInput") for v in VARS}
    wqk_d = {v: nc.dram_tensor(f"wqk{v}", [128, KC, 8, 128], F8,
                               kind="ExternalInput") for v in VARS}
    wv_d = {v: nc.dram_tensor(f"wv{v}", [128, KC, 512], F8,
                              kind="ExternalInput") for v in VARS}
    wproj_d = nc.dram_tensor("wproj", [128, 4, 1024], BF16,
                             kind="ExternalInput")
    bqk_d = nc.dram_tensor("bqk", [128, 8], F32, kind="ExternalInput")
    bv_d = nc.dram_tensor("bv", [128, 512], F32, kind="ExternalInput")
    tri_d = nc.dram_tensor("tri", [128, 128], BF16, kind="ExternalInput")
    out_d = nc.dram_tensor("outT", [128, 8, t], BF16, kind="ExternalOutput")

    # ---- schedule state ----
    prog = {"sync": [], "tensor": [], "vector": [], "scalar": []}
    cnt = {"pe": 0, "act": 0, "dve": 0}
    for _c in range(8):
        cnt[f"dma{_c}"] = 0
    last_wait = {e: {} for e in prog}
    bank_war = {}          # psum bank -> (sem, value): last consumer done
    FUSE = {"tensor", "vector", "scalar"}

    def op(engine, fn, waits=(), incs=()):
        w = []
        for s, v in waits:
            if v <= 0 or last_wait[engine].get(s, -1) >= v:
                continue
            last_wait[engine][s] = v
            w.append((s, v))
        prog[engine].append((fn, w, list(incs), engine in FUSE))
        for s, a in incs:
            cnt[s] += a

    NDMA = 8
    dma_rr = [0]

    def dma(dst, src, waits=()):
        ch = dma_rr[0] % NDMA
        dma_rr[0] += 1
        sem = f"dma{ch}"
        w = [(sem, cnt[sem])] + list(waits)   # chain within channel
        op("sync", lambda e, d=dst, s=src: e.dma_start(d, s),
           w, [(sem, 16)])
        return (sem, cnt[sem])

    stack = contextlib.ExitStack()
    sb = lambda name, shape, dt: stack.enter_context(
        nc.sbuf_tensor(name, shape, dt))

    # persistent region (x / wv stay resident for the wavefront A2)
    qk_sb = sb("qk", [128, 8, t], BF16)
    v_sb = sb("vsb", [128, tt_n, 8, 65], BF16)
    bqk_sb = sb("bqk_sb", [128, 8], F32)
    bv_sb = sb("bv_sb", [128, 512], F32)
    tri_sb = sb("tri_sb", [128, 128], BF16)
    x_sb = {v: sb(f"x{v}_sb", [128, KC, t], F8) for v in VARS}
    wv_sb = {v: sb(f"wv{v}_sb", [128, KC, 512], F8) for v in VARS}
    psum = stack.enter_context(nc.psum_tensor("ps", [128, 8, 512], F32))

    with contextlib.ExitStack() as semstack:
        semstack.enter_context(nc.allow_low_precision(
            reason="bf16/fp8 matmul operands and bf16 recip intentional"))
        sems = {}
        for _nm in ["pe", "act", "dve"] + [f"dma{_c}" for _c in range(8)]:
            sems[_nm] = semstack.enter_context(nc.semaphore(_nm + "_sem"))

        bqk_ret = dma(bqk_sb.ap(), bqk_d[:])
        bv_ret = dma(bv_sb.ap(), bv_d[:])
        tri_ret = dma(tri_sb.ap(), tri_d[:])

        # ones column of V_aug via DVE (x*0 + 1)
        op("vector",
           lambda e: e.tensor_scalar(
               v_sb.ap()[:, :, :, 64:65],
               bv_sb.ap()[:, 0:tt_n * 8].rearrange(
                   "p (a b c) -> p a b c", a=tt_n, b=8),
               0.0, 1.0, MULT, mybir.AluOpType.add),
           [bv_ret], [("dve", 1)])
        vones_ret = ("dve", cnt["dve"])

        # wqk region (A1 only; aliased by phase-B/C tiles afterwards)
        wq_ctx = []
        wqk_sb = {}
        for v in VARS:
            c = nc.sbuf_tensor(f"wqk{v}_sb", [128, KC, 8, 128], F8)
            wqk_sb[v] = c.__enter__()
            wq_ctx.append(c)

        first_pv = [True]
        pair_war = {0: 0, 1: 0}
        slot_war = {0: 0, 1: 0}
        ybank_war = {}
        rsb_war = {}        # buf -> bcast dma ret (WAR for recip write)
        rep_war = {}        # buf -> dve cnt of norm (WAR for bcast write)
        ysbt_war = {}       # g -> shift dma ret (WAR for staging slot)
        pending_tail = []
        pending_pv = []
        b_alloc = [None]
        c_copy = {}
        c_dma = {}
        out_seq = [0]
        rep_gate = []

        for rep in range(reps):
            # ---- phase A DMAs (whole tensors; T1's land first) ----
            xdma = {}
            wqdma = {}
            wvdma = {}
            xdma["h"] = dma(x_sb["h"].ap(), x_d["h"][:], list(rep_gate))
            wqdma["h"] = dma(wqk_sb["h"].ap(), wqk_d["h"][:],
                             list(rep_gate))
            wqdma["l"] = dma(wqk_sb["l"].ap(), wqk_d["l"][:],
                             list(rep_gate))
            xdma["l"] = dma(x_sb["l"].ap(), x_d["l"][:], list(rep_gate))
            for v_ in VARS:
                wvdma[v_] = dma(wv_sb[v_].ap(), wv_d[v_][:],
                                list(rep_gate))

            # ---- phase A1: qk^T = w_qk^T @ x^T (fp8 DR 3-term) ----
            for grp in range(2):
                for tc in range(tc_n):
                    done = {}
                    for ti, (wvar, xvar) in enumerate(A_TERMS):
                        for kp in range(4):
                            for ftl in range(4):
                                bank = (tc % 2) * 4 + ftl
                                ft = grp * 4 + ftl
                                w = list(rep_gate)
                                if ftl == 0:
                                    w += [xdma[xvar], wqdma[wvar]]
                                if kp == 0 and ti == 0 \
                                        and bank in bank_war:
                                    w.append(bank_war.pop(bank))
                                last = (kp == 3 and ti == 2)
                                op("tensor",
                                   lambda e, b=bank, f=ft, k=kp, tc_=tc,
                                   wv_=wvar, xv_=xvar, st=(
                                       kp == 0 and ti == 0), sp=last:
                                       e.matmul(
                                           psum.ap()[:, b],
                                           wqk_sb[wv_].ap()[
                                               :, 2 * k:2 * k + 2, f],
                                           x_sb[xv_].ap()[
                                               :, 2 * k:2 * k + 2,
                                               tc_ * TQ:(tc_ + 1) * TQ],
                                           start=st, stop=sp,
                                           perf_mode=DRM),
                                   w, [("pe", 1)] if last else [])
                                if last:
                                    done[ftl] = cnt["pe"]
                    dsc = 1.0 / (A_SC * (BQ_SC if grp == 0 else BK_SC))
                    for ftl in range(4):
                        ft = grp * 4 + ftl
                        bk = (tc % 2) * 4 + ftl
                        op("vector",
                           lambda e, b=bk, f=ft, tc_=tc, c=dsc:
                               e.tensor_scalar(
                                   qk_sb.ap()[:, f, tc_ * TQ:(tc_ + 1) * TQ],
                                   psum.ap()[:, b],
                                   c, bqk_sb.ap()[:, f:f + 1], MULT, ADD),
                           [("pe", done[ftl]), bqk_ret], [("dve", 1)])
                        bank_war[bk] = ("dve", cnt["dve"])
            a1_copies = cnt["dve"]
            a1_pe_done = cnt["pe"]

            # ---- A2 helper: one V t-tile (psum banks 6/7) ----
            a2_done = {}

            def emit_a2_tile(tg, j):
                tt = 4 * tg + j
                bank = 6 + j % 2
                done = 0
                for ti, (wvar, xvar) in enumerate(A_TERMS):
                    for kp in range(4):
                        w = list(rep_gate)
                        w += [wvdma[wvar], xdma[xvar]]
                        if kp == 0 and ti == 0 and bank in bank_war:
                            w.append(bank_war.pop(bank))
                        last = (kp == 3 and ti == 2)
                        op("tensor",
                           lambda e, b=bank, k=kp, tt_=tt,
                           wv_=wvar, xv_=xvar,
                           st=(kp == 0 and ti == 0), sp=last:
                               e.matmul(
                                   psum.ap()[:, b],
                                   x_sb[xv_].ap()[
                                       :, 2 * k:2 * k + 2,
                                       tt_ * 128:(tt_ + 1) * 128],
                                   wv_sb[wv_].ap()[
                                       :, 2 * k:2 * k + 2],
                                   start=st, stop=sp,
                                   perf_mode=DRM),
                           w, [("pe", 1)] if last else [])
                        if last:
                            done = cnt["pe"]
                op("vector",
                   lambda e, b=bank, tt_=tt:
                       e.scalar_tensor_tensor(
                           v_sb.ap()[:, tt_, :, 0:64],
                           psum.ap()[:, b], 1.0 / (A_SC * BV_SC),
                           bv_sb.ap()[:], MULT, ADD),
                   [("pe", done), bv_ret], [("dve", 1)])
                bank_war[bank] = ("dve", cnt["dve"])
                if j == 3:
                    a2_done[tg] = ("dve", cnt["dve"])

            if b_alloc[0] is None:
                for _a in reversed(wq_ctx):
                    _a.__exit__(None, None, None)
                ysb = sb("ysb", [128, 4, t], BF16)
                ysbt = sb("ysbt", [64, 4, 512], BF16)
                pt_sb = sb("pt", [128, 4, 512], BF16)
                yun = sb("yun", [64, 2, 512], BF16)
                rsb = sb("rsb", [65, 2, 512], BF16)
                rep_sb = sb("repb", [64, 2, 512], BF16)
                osb = sb("osb", [128, 8, 512], BF16)
                wproj_sb = sb("wproj_sb", [128, 4, 1024], BF16)
                b_alloc[0] = (ysb, ysbt, pt_sb, yun, rsb, rep_sb, osb,
                              wproj_sb)
            else:
                (ysb, ysbt, pt_sb, yun, rsb, rep_sb, osb,
                 wproj_sb) = b_alloc[0]

            # wproj may alias the wqk region: wait for A1's last read
            wproj_dma = dma(wproj_sb.ap(), wproj_d[:],
                            [("pe", a1_pe_done)])

            for _j in range(4):
                emit_a2_tile(0, _j)

            # ---- phase C helpers (one wave = 8 output tiles) ----
            def c_mm(bank, g_, ft, tc_, st, sp):
                return lambda e, bk=bank, g=g_, f=ft, tc__=tc_, s=st, p=sp: \
                    e.matmul(
                        psum.ap()[:, bk],
                        wproj_sb.ap()[:, g, f * 128:(f + 1) * 128],
                        ysb.ap()[:, g, tc__ * TQ:(tc__ + 1) * TQ],
                        start=s, stop=p)

            def c_tile_start(j, bank, ft, tc_, ydone, act_fence):
                # act_fence: last exp of this wave reading psum banks 0..3
                w = [wproj_dma, ("act", act_fence)]
                if j >= 4:
                    w.append(("act", c_copy[j - 4]))
                if bank in bank_war:
                    w.append(bank_war.pop(bank))
                for g_ in range(3):
                    op("tensor", c_mm(bank, g_, ft, tc_, g_ == 0, False),
                       (w + ydone[g_]) if g_ == 0 else ydone[g_], [])

            def c_tile_end(j, bank, ft, tc_, ydone):
                op("tensor", c_mm(bank, 3, ft, tc_, False, True),
                   ydone[3], [("pe", 1)])
                mm_done = cnt["pe"]
                w = [("pe", mm_done)]
                if j >= 8:
                    w.append(c_dma[j - 8])
                op("scalar",
                   lambda e, bk=bank, ob=j % 8: e.activation(
                       osb.ap()[:, ob], psum.ap()[:, bk], COPY),
                   w, [("act", 1)])
                c_copy[j] = cnt["act"]
                bank_war[bank] = ("act", cnt["act"])
                c_dma[j] = dma(
                    out_d[:, ft, tc_ * TQ:(tc_ + 1) * TQ],
                    osb.ap()[:, j % 8],
                    [("act", c_copy[j])])

            # ---- waves: B(all heads, qc) + A2(tg qc+1) + C(tc=qc) ----
            for qc in range(tc_n):
                ydone = {}        # g -> waits for this wave's C
                for hi in range(NH):
                    h = hi
                    g = h // 2
                    qrow = (h % 2) * 64
                    qf, kf = g, 4 + g
                    i = qc * NH + hi
                    yb = 4 + i % 2
                    nkt = 4 * qc + 4
                    npairs = 2 * qc + 2

                    def s_mm(kt, bank, qrow=qrow, kf=kf, qf=qf, qc=qc):
                        r = kt - 4 * qc
                        off = max(0, r * 128)
                        n = TQ - off
                        return lambda e, kt=kt, b=bank, off=off, n=n: \
                            e.matmul(
                                psum.ap()[:, b, off:off + n],
                                qk_sb.ap()[qrow:qrow + 64, kf,
                                           kt * 128:(kt + 1) * 128],
                                qk_sb.ap()[qrow:qrow + 64, qf,
                                           qc * TQ + off:qc * TQ + off + n],
                                start=True, stop=True)

                    def pv_mm(kt, slot, start, stop, h=h, qc=qc, yb=yb):
                        r = kt - 4 * qc
                        off = max(0, r * 128)
                        n = TQ - off
                        return lambda e, kt=kt, s=slot, off=off, n=n, \
                            st=start, sp=stop: e.matmul(
                                psum.ap()[0:65, yb, off:off + n],
                                v_sb.ap()[:, kt, h, :],
                                pt_sb.ap()[:, s, off:off + n],
                                start=st, stop=sp)

                    s_done = {}
                    pt_ready = {}

                    for p in range(npairs):
                        pg = p % 2
                        kts = (2 * p, 2 * p + 1)
                        banks = (pg * 2, pg * 2 + 1)
                        w = [("act", pair_war[pg]), ("dve", a1_copies)]
                        for bq in banks:
                            if bq in bank_war:
                                w.append(bank_war.pop(bq))
                        op("tensor", s_mm(kts[0], banks[0]), w, [])
                        op("tensor", s_mm(kts[1], banks[1]), [],
                           [("pe", 1)])
                        s_done[p] = cnt["pe"]
                        if p == 0:
                            # flush the previous iteration's deferred final
                            # PV (+recip/bcast), then the norms whose bcasts
                            # were issued one iteration ago
                            tails_old = pending_tail[:]
                            pending_tail.clear()
                            for _f in pending_pv:
                                _f()
                            pending_pv.clear()
                            for _t in tails_old:
                                _t()
                        if p >= 1:
                            pp = p - 1
                            w = [pt_ready[pp], a2_done[qc]]
                            if first_pv[0]:
                                w += [vones_ret]
                                first_pv[0] = False
                            if pp == 0 and yb in ybank_war:
                                w.append(("dve", ybank_war[yb]))
                            op("tensor",
                               pv_mm(2 * pp, (pp % 2) * 2,
                                     2 * pp == 0, False), w, [])
                            op("tensor",
                               pv_mm(2 * pp + 1, (pp % 2) * 2 + 1, False,
                                     2 * pp + 1 == nkt - 1),
                               [], [("pe", 1)])
                            slot_war[pp % 2] = cnt["pe"]
                        # exp; diag pair split to skip dead columns
                        diag = (kts[1] - 4 * qc) >= 0
                        r0 = kts[0] - 4 * qc
                        w = [("pe", s_done[p]), ("pe", slot_war[pg])]
                        if diag and r0 >= 2:
                            op("scalar",
                               lambda e, bq=banks[0], s=pg * 2, o=128 * r0:
                                   e.activation(
                                       pt_sb.ap()[:, s, o:TQ],
                                       psum.ap()[:, bq, o:TQ], EXP),
                               w, [])
                            op("scalar",
                               lambda e, bq=banks[1], s=pg * 2 + 1,
                               o=128 * (r0 + 1):
                                   e.activation(
                                       pt_sb.ap()[:, s, o:TQ],
                                       psum.ap()[:, bq, o:TQ], EXP),
                               [], [("act", 1)])
                        else:
                            off0 = max(0, r0) * 128
                            op("scalar",
                               lambda e, bq=banks[0], s=pg * 2, o=off0:
                                   e.activation(
                                       pt_sb.ap()[:, s:s + 2]
                                           .rearrange("p a b -> p (a b)")
                                           [:, o:2 * TQ],
                                       psum.ap()[:, bq:bq + 2]
                                           .rearrange("p a b -> p (a b)")
                                           [:, o:2 * TQ],
                                       EXP),
                               w, [("act", 1)])
                        pair_war[pg] = cnt["act"]
                        pt_ready[p] = ("act", cnt["act"])
                        if diag:
                            for j in (0, 1):
                                r = kts[j] - 4 * qc
                                if r < 0:
                                    continue
                                op("vector",
                                   lambda e, s=pg * 2 + j, r=r:
                                       e.tensor_tensor(
                                           pt_sb.ap()[:, s,
                                                      r * 128:r * 128 + 128],
                                           pt_sb.ap()[:, s,
                                                      r * 128:r * 128 + 128],
                                           tri_sb.ap()[:], MULT),
                                   [("act", pt_ready[p][1]), tri_ret],
                                   [("dve", 1)] if j == 1 else [])
                            pt_ready[p] = ("dve", cnt["dve"])

                    buf = i % 2
                    if h % 2 == 0:
                        out_ap = ysb.ap()[0:64, g, qc * TQ:(qc + 1) * TQ]
                    else:
                        out_ap = ysbt.ap()[0:64, g, :]

                    def _tail(out_ap, buf, bret, h, g, qc, ydone):
                        def emit():
                            w = [bret]
                            if h % 2 == 1 and g in ysbt_war:
                                w.append(ysbt_war[g])
                            op("vector",
                               lambda e, o=out_ap, b=buf:
                                   e.tensor_tensor(
                                       o, yun.ap()[0:64, b],
                                       rep_sb.ap()[0:64, b], MULT),
                               w, [("dve", 1)])
                            rep_war[buf] = cnt["dve"]
                            if h % 2 == 1:
                                nd = dma(
                                    ysb.ap()[64:128, g,
                                             qc * TQ:(qc + 1) * TQ],
                                    ysbt.ap()[0:64, g, :],
                                    [("dve", cnt["dve"])])
                                ysbt_war[g] = nd
                                ydone.setdefault(g, []).append(nd)
                            else:
                                ydone.setdefault(g, []).append(
                                    ("dve", cnt["dve"]))
                        return emit

                    def _pv_final(pp=npairs - 1, ptr=pt_ready[npairs - 1],
                                  pv_mm=pv_mm, nkt=nkt, yb=yb, buf=buf,
                                  out_ap=out_ap, h=h, g=g, qc=qc,
                                  ydone=ydone):
                        def emit():
                            w = [ptr, a2_done[qc]]
                            if first_pv[0]:
                                w += [vones_ret]
                                first_pv[0] = False
                            op("tensor", pv_mm(2 * pp, (pp % 2) * 2,
                                               False, False), w, [])
                            op("tensor",
                               pv_mm(2 * pp + 1, (pp % 2) * 2 + 1,
                                     False, True), [], [("pe", 1)])
                            slot_war[pp % 2] = cnt["pe"]
                            pv_all = cnt["pe"]
                            # reciprocal of sums (row 64) -> bf16
                            w = [("pe", pv_all)]
                            if buf in rsb_war:
                                w.append(rsb_war[buf])
                            op("vector",
                               lambda e, yb=yb, b=buf: e.reciprocal(
                                   rsb.ap()[64:65, b],
                                   psum.ap()[64:65, yb]),
                               w, [("dve", 1)])
                            recip_done = cnt["dve"]
                            op("vector",
                               lambda e, yb=yb, b=buf: e.tensor_copy(
                                   yun.ap()[0:64, b], psum.ap()[0:64, yb]),
                               [], [("dve", 1)])
                            ybank_war[yb] = cnt["dve"]
                            # broadcast recip across 64 partitions via DMA
                            bw = [("dve", recip_done)]
                            if buf in rep_war:
                                bw.append(("dve", rep_war[buf]))
                            bsrc = rsb.ap()[64:65, buf] \
                                .rearrange("p (a n) -> p a n", a=1) \
                                .broadcast_to([1, 64, TQ])
                            bcast_ret = dma(rep_sb.ap()[0:64, buf],
                                            bsrc, bw)
                            rsb_war[buf] = bcast_ret
                            pending_tail.append(
                                _tail(out_ap, buf, bcast_ret, h, g, qc,
                                      ydone))
                        return emit
                    pending_pv.append(_pv_final())

                    # spread next wave's A2 tiles through this wave so
                    # thin iterations have PE filler over the exp latency
                    if qc + 1 < tc_n and hi in (1, 3, 5):
                        emit_a2_tile(qc + 1, (hi - 1) // 2)

                # flush the last deferred PV first (its recip/bcast then
                # drain under the last A2 tile's PE block), then the norms
                tails_old = pending_tail[:]
                pending_tail.clear()
                for _f in pending_pv:
                    _f()
                pending_pv.clear()
                if qc + 1 < tc_n:
                    emit_a2_tile(qc + 1, 3)
                for _t in tails_old:
                    _t()
                for _t in pending_tail:
                    _t()
                pending_tail.clear()

                # ---- phase C for this wave ----
                yd = {g_: ydone[g_] for g_ in range(4)}
                b_act_wave = cnt["act"]
                defer = 2
                pend = []
                for ft in range(8):
                    j = out_seq[0] + ft
                    bank = j % 4
                    c_tile_start(j, bank, ft, qc, yd, b_act_wave)
                    if ft < defer:
                        pend.append((j, bank, ft, qc))
                        continue
                    while pend:
                        c_tile_end(*pend.pop(0), yd)
                    c_tile_end(j, bank, ft, qc, yd)
                while pend:
                    c_tile_end(*pend.pop(0), yd)
                out_seq[0] += 8

            rep_gate = [("act", c_copy[out_seq[0] - 1]),
                        c_dma[out_seq[0] - 1]]
            # seed psum WARs for next rep's A phase
            for bk in (4, 5):
                bank_war.setdefault(bk, ("dve", ybank_war.get(bk, 0)))

        # ---- emit ----
        with nc.Block() as block:
            def emitter(name):
                def run(eng):
                    for fn, waits, incs, fuse in prog[name]:
                        pre = waits[1:] if (fuse and waits) else waits
                        for s, v in pre:
                            eng.wait_ge(sems[s], v)
                        ins = fn(eng)
                        if fuse and waits:
                            s, v = waits[0]
                            ins.wait_op(sems[s], v, "sem-ge")
                        for s, a in incs:
                            ins.then_inc(sems[s], a)
                return run
            block.sync(emitter("sync"))
            block.tensor(emitter("tensor"))
            block.vector(emitter("vector"))
            block.scalar(emitter("scalar"))

    stack.close()
    return nc


# ---------------------------------------------------------------------------

def _f8(v):
    return np.ascontiguousarray(v).astype(NF8)


def _split8(v, hi_sc):
    """Return (hi, lo) fp8 arrays for scaled 3-term matmul."""
    hi = _f8(hi_sc * v)
    lo = _f8(hi_sc * v - hi.astype(np.float32))
    return hi, lo


def host_prep(x, W_qkv, b_qkv, W_proj, b_proj, t=T):
    scale = 1.0 / math.sqrt(D_K)
    x = np.asarray(x, np.float32)
    W_qkv = np.asarray(W_qkv, np.float32)
    b_qkv = np.asarray(b_qkv, np.float32)
    W_proj = np.asarray(W_proj, np.float32)

    tri = (np.arange(128)[None, :] >= np.arange(128)[:, None]) \
        .astype(NBF)

    in_maps = []
    for c in range(N_CORES):
        b = c // 2
        f0 = (c % 2) * 512
        xT = np.ascontiguousarray(
            x[b, :t].T.reshape(KC, 128, t).transpose(1, 0, 2))
        xh, xl = _split8(xT, A_SC)

        wq = W_qkv[:, f0:f0 + 512] * scale
        wk = W_qkv[:, D_MODEL + f0:D_MODEL + f0 + 512]

        def wlayout(w):  # [1024, 512] -> [128, KC, 4, 128]
            return np.ascontiguousarray(
                w.reshape(KC, 128, 4, 128).transpose(1, 0, 2, 3))

        wqk = {}
        qh, ql = _split8(wlayout(wq), BQ_SC)
        kh, kl = _split8(wlayout(wk), BK_SC)
        wqk["h"] = np.concatenate([qh, kh], axis=2)
        wqk["l"] = np.concatenate([ql, kl], axis=2)

        wv = W_qkv[:, 2 * D_MODEL + f0:2 * D_MODEL + f0 + 512]
        wv = np.ascontiguousarray(
            wv.reshape(KC, 128, 512).transpose(1, 0, 2))
        vh, vl = _split8(wv, BV_SC)

        bq = b_qkv[f0:f0 + 512] * scale
        bk_ = b_qkv[D_MODEL + f0:D_MODEL + f0 + 512]
        bqk = np.ascontiguousarray(
            np.concatenate([bq, bk_]).reshape(8, 128).T).astype(np.float32)
        bv = b_qkv[2 * D_MODEL + f0:2 * D_MODEL + f0 + 512]
        bv_rep = np.broadcast_to(bv, (128, 512)).astype(np.float32).copy()
        wp = W_proj[f0:f0 + 512]
        wp = np.ascontiguousarray(
            wp.reshape(4, 128, 1024).transpose(1, 0, 2)).astype(NBF)
        in_maps.append({
            "xh": xh, "xl": xl,
            "wqkh": wqk["h"], "wqkl": wqk["l"],
            "wvh": vh, "wvl": vl,
            "wproj": wp, "bqk": bqk, "bv": bv_rep, "tri": tri,
        })
    return in_maps


def host_gather(results, b_proj, t=T):
    b_proj = np.asarray(b_proj, np.float32)
    out = np.empty((B, t, D_MODEL), np.float32)
    for b in range(B):
        acc = None
        for half in range(2):
            r = results[2 * b + half]["outT"].astype(np.float32)
            oT = r.transpose(1, 0, 2).reshape(D_MODEL, t)
            acc = oT if acc is None else acc + oT
        out[b] = acc.T + b_proj
    return out


_NC_CACHE = {}


def kernel(x, W_qkv, b_qkv, W_proj, b_proj):
    if T not in _NC_CACHE:
        _NC_CACHE[T] = build_nc(T)
    nc = _NC_CACHE[T]
    in_maps = host_prep(x, W_qkv, b_qkv, W_proj, b_proj)
    res = run_bass_kernel_spmd(nc, in_maps, core_ids=list(range(N_CORES)))
    return host_gather(res.results, b_proj)
